# revision 13
# baseline (speedup 1.0000x reference)
"""Slot-attention (softmax over queries + key renormalization) on 8 TRN2 NeuronCores.

Sharding: data-parallel over batch (b=8 -> one batch element per core, no
collectives). Per-core fused kernel, redesigned around two ideas:

1. All input transposes (x, context, Wq/Wk/Wv/Wo) go through the DMA XBAR
   (dma_start(transpose=True), 16-bit) instead of PE identity-matmuls:
   f32 DMA load -> DVE/Pool convert to bf16 -> one XBAR transpose per
   [128,512] source tile directly into the blocked SBUF layout. PE does
   zero transpose work for inputs.

2. attn@v is computed in the [i, e] orientation with the exp tiles as the
   STATIONARY operand: out[i-tile, 64] += expT[jt][:, i-slice]^T @ vhs[jt].
   Contraction is the full 128 j-rows (vs 64 e-rows in the [e, i]
   orientation), halving PE cycles for attn@v. A 1-column matmul per
   (it, jt) accumulates the renormalizer r[i] into a persistent PSUM tile.
   Normalization attn/(r[i]) becomes a per-partition scalar multiply
   (Pool), and the normalized [i, e]-tiles are transposed back to [e, i]
   for the output projection with XBAR transposes as well.

Pipeline: per head h, per j-tile jt: sim (PE) -> exp with S[j] accum (ACT)
-> [one step later] invS column recip + vhs (Pool) -> 16 av/r matmuls (PE).
Prep work (weight streams, v/q/k projections, output projections of closed
head-pairs) is interleaved as budget-drained filler generators.

Matmul operands bf16, accumulation f32 in PSUM, softmax statistics f32.
"""

import os
import sys

sys.path.insert(0, "/opt/trn_rl_repo")

import numpy as np

import concourse.bass as bass
import concourse.mybir as mybir
import concourse.tile as tile
from concourse import bacc
from concourse.bass_utils import run_bass_kernel_spmd

B = 8
N = 1024  # queries
M = 1024  # keys
D = 512   # model dim
H = 8
DH = 64
INNER = H * DH
SCALE = DH ** -0.5
P = 128

F32 = mybir.dt.float32
CDT = mybir.dt.bfloat16

AV0LAG = int(os.environ.get("AV0LAG", "4"))
FB = int(os.environ.get("FB", "3"))
_DRAINED = {}

Exp = mybir.ActivationFunctionType.Exp
Mult = mybir.AluOpType.mult
Add = mybir.AluOpType.add


def _r3(ap, a):
    """View a [P, a*b] AP as [P, a, b]."""
    return ap.rearrange("p (a b) -> p a b", a=a)


def build(nc: bass.Bass):
    _DRAINED.clear()
    x_d = nc.declare_dram_parameter("x", [N, D], F32, isOutput=False)
    c_d = nc.declare_dram_parameter("context", [M, D], F32, isOutput=False)
    wq_d = nc.declare_dram_parameter("Wq", [INNER, D], F32, isOutput=False)
    wk_d = nc.declare_dram_parameter("Wk", [INNER, D], F32, isOutput=False)
    wv_d = nc.declare_dram_parameter("Wv", [INNER, D], F32, isOutput=False)
    wo_d = nc.declare_dram_parameter("Wo", [D, INNER], F32, isOutput=False)
    bo_d = nc.declare_dram_parameter("bo", [D], F32, isOutput=False)
    out_d = nc.declare_dram_parameter("out", [N, D], F32, isOutput=True)

    with tile.TileContext(nc) as tc:
        with tc.tile_pool(name="const", bufs=1) as const:
            ones128 = const.tile([1, P], CDT, tag="ones128")
            nc.gpsimd.memset(ones128[:, :], 1.0)
            # tiny warmup exp so the ACT table load happens at t~0
            warm = const.tile([1, 2], F32, tag="warm")
            nc.gpsimd.memset(warm[:, :], 0.0)
            nc.scalar.activation(warm[:, 0:1], warm[:, 1:2], Exp, scale=1.0)
            bo_s = const.tile([1, D], F32, tag="bo_s")
            bo_sb = const.tile([1, D], CDT, tag="bo_sb")
            bo_b = const.tile([P, D], F32, tag="bo_b")

            with tc.tile_pool(name="stage", bufs=1) as stage:
                # wT[n][p, (et*4+dt)*128 + f] = W[et*128+f, dt*128+p]
                wT = {n: stage.tile([P, 16 * P], CDT, tag=f"w{n}T", name=f"w{n}T")
                      for n in ("q", "k", "v")}
                # woT[p, (dt*4+et)*128 + f] = Wo[dt*128+f, et*128+p]
                woT = stage.tile([P, 16 * P], CDT, tag="woT")
                # xT_b[p, (nt*4+dt)*128 + f] = x[nt*128+f, dt*128+p]
                xT = stage.tile([P, 32 * P], CDT, tag="xT")
                cT = stage.tile([P, 32 * P], CDT, tag="cT")
                v = [stage.tile([P, INNER], CDT, tag=f"v{t}", name=f"v{t}")
                     for t in range(8)]

                with tc.tile_pool(name="outp", bufs=1) as outp:
                    qT = [outp.tile([P, N], CDT, tag=f"qT{t}", name=f"qT{t}") for t in range(4)]
                    kT = [outp.tile([P, M], CDT, tag=f"kT{t}", name=f"kT{t}") for t in range(4)]
                    outT = [outp.tile([P, N], CDT, tag=f"outT{t}", name=f"outT{t}") for t in range(4)]
                    y_acc = [outp.tile([P, D], F32, tag=f"y_acc{t}", name=f"y_acc{t}")
                             for t in range(8)]

                    with tc.tile_pool(name="head", bufs=1) as head, \
                         tc.tile_pool(name="norm", bufs=1) as norm, \
                         tc.tile_pool(name="ld", bufs=1) as ld:
                        ps_at = tc.alloc_tile_pool(name="ps_at", bufs=1, space="PSUM")
                        ps_rp = tc.alloc_tile_pool(name="ps_rp", bufs=1, space="PSUM")
                        r_all = ps_rp.tile([P, 64], F32, tag="r_all")
                        st = _State(nc, tc, head, norm, ld, ps_at, r_all,
                                    wT, woT, xT, cT, v, qT, kT, outT, y_acc,
                                    ones128, bo_s, bo_sb, bo_b,
                                    x_d, c_d, wq_d, wk_d, wv_d, wo_d, bo_d, out_d)
                        _run(st)
                        ps_rp.release()
                        ps_at.release()
    return nc


class _State:
    def __init__(self, nc, tc, head, norm, ld, ps_at, r_all,
                 wT, woT, xT, cT, v, qT, kT, outT, y_acc,
                 ones128, bo_s, bo_sb, bo_b,
                 x_d, c_d, wq_d, wk_d, wv_d, wo_d, bo_d, out_d):
        self.nc = nc
        self.tc = tc
        self.head = head
        self.norm = norm
        self.ld = ld
        self.ps = ps_at
        self.r_all = r_all
        self.wT = wT
        self.woT = woT
        self.xT = xT
        self.cT = cT
        self.v = v
        self.qT = qT
        self.kT = kT
        self.outT = outT
        self.y_acc = y_acc
        self.ones128 = ones128
        self.bo_s = bo_s
        self.bo_sb = bo_sb
        self.bo_b = bo_b
        self.x_d = x_d
        self.c_d = c_d
        self.w_d = {"q": wq_d, "k": wk_d, "v": wv_d}
        self.wo_d = wo_d
        self.bo_d = bo_d
        self.out_d = out_d
        self.fillers = []     # FIFO of (name, generator)
        self.wb = {}          # bf16 staging tiles for weights
        self.invr = {}        # per-head [P, 8] f32 reciprocal renormalizers
        self.avn = {}         # (pair, half) -> [P, 512] CDT normalized tiles


def _drain(g):
    if g is not None:
        for _ in g:
            pass


def _budget_drain(st, budget):
    while budget > 0 and st.fillers:
        try:
            next(st.fillers[0][1])
            _DRAINED[st.fillers[0][0]] = _DRAINED.get(st.fillers[0][0], 0) + 1
            budget -= 1
        except StopIteration:
            st.fillers.pop(0)


def _force(st, name):
    """Fully drain filler `name` (and everything before it in FIFO order
    stays untouched -- only the named one is searched and drained)."""
    for pair in list(st.fillers):
        if pair[0] == name:
            _drain(pair[1])
            st.fillers.remove(pair)


def _force_until(st, name, count):
    """Drain the FIFO head while it is `name` until its drained-counter
    reaches `count` (used for emission-order gating, e.g. vproj groups)."""
    while st.fillers and st.fillers[0][0] == name and \
            _DRAINED.get(name, 0) < count:
        try:
            next(st.fillers[0][1])
            _DRAINED[name] = _DRAINED.get(name, 0) + 1
        except StopIteration:
            st.fillers.pop(0)


# ---------------------------------------------------------------- prep ----

def _xbar(nc, dst3, src2):
    nc.sync.dma_start(dst3, src2, transpose=True)


def _load_xc_pair(st, which, k):
    """DMA one [128, 1024] f32 pair (tiles 2k, 2k+1) of x or context."""
    nc = st.nc
    src = st.x_d if which == "x" else st.c_d
    sb = st.ld.tile([P, 1024], F32, tag=f"{which}ld", bufs=3, name=f"{which}ld{k}")
    nc.sync.dma_start(
        sb[:, :],
        src[2 * k * P:(2 * k + 2) * P, :].rearrange("(t p) d -> p t d", p=P))
    return sb


def _conv_xbar_xc(st, which, k, sb, eng=None):
    """Convert pair k to bf16 and XBAR it into xT_b/cT_b."""
    nc = st.nc
    dstT = st.xT if which == "x" else st.cT
    bf = st.ld.tile([P, 1024], CDT, tag=f"{which}b", bufs=3, name=f"{which}b{k}")
    engs = eng or (nc.vector, nc.vector)
    engs[0].tensor_copy(bf[:, 0:512], sb[:, 0:512])
    engs[1].tensor_copy(bf[:, 512:1024], sb[:, 512:1024])
    _xbar(nc, _r3(dstT[:, 8 * k * P:(8 * k + 8) * P], 8), bf[:, :])


def _load_w(st, name):
    """DMA a q/k/v weight matrix as [128, 2048] (4 e-tiles side by side)."""
    nc = st.nc
    sb = st.ld.tile([P, 2048], F32, tag="wld", bufs=3, name=f"w{name}ld")
    nc.sync.dma_start(
        sb[:, :],
        st.w_d[name][:, :].rearrange("(t p) d -> p t d", p=P))
    return sb


def _conv_xbar_w(st, name, sb, ets, eng):
    """Convert e-tiles `ets` of weight `name` to bf16 and XBAR into wT."""
    nc = st.nc
    for et in ets:
        bf = st.ld.tile([P, 512], CDT, tag="wbc", bufs=3, name=f"wb{name}{et}")
        eng.tensor_copy(bf[:, :], sb[:, et * 512:(et + 1) * 512])
        _xbar(nc, _r3(st.wT[name][:, et * 512:(et + 1) * 512], 4), bf[:, :])
        yield


def _q_proj(st, et, ic):
    """qT[et][:, ic*512:(ic+1)*512] = sum_dt wqT(dt,et)^T @ xT_b(dt, ic-half)."""
    nc = st.nc
    pp = st.ps.tile([P, 512], F32, tag="ps_misc", bufs=2, name=f"pq{et}_{ic}")
    xr = st.xT[:, :].rearrange("p (nt dt f) -> p dt nt f", nt=8, dt=4)
    for dt in range(4):
        nc.tensor.matmul(
            pp[:, :],
            st.wT["q"][:, (et * 4 + dt) * P:(et * 4 + dt + 1) * P],
            xr[:, dt, 4 * ic:4 * ic + 4, :],
            start=(dt == 0), stop=(dt == 3))
        yield
    nc.vector.tensor_copy(st.qT[et][:, ic * 512:(ic + 1) * 512], pp[:, :])


def _k_proj_chunk(st, et, k):
    """kT[et][:, k*256:(k+1)*256] from context pair k (2 j-tiles)."""
    nc = st.nc
    pk = st.ps.tile([P, 256], F32, tag="ps_misc", bufs=2, name=f"pk{et}_{k}")
    cr = st.cT[:, :].rearrange("p (nt dt f) -> p dt nt f", nt=8, dt=4)
    for dt in range(4):
        nc.tensor.matmul(
            pk[:, :],
            st.wT["k"][:, (et * 4 + dt) * P:(et * 4 + dt + 1) * P],
            cr[:, dt, 2 * k:2 * k + 2, :],
            start=(dt == 0), stop=(dt == 3))
        yield
    nc.vector.tensor_copy(st.kT[et][:, k * 256:(k + 1) * 256], pk[:, :])


def _k_proj(st, et, ic):
    nc = st.nc
    pk = st.ps.tile([P, 512], F32, tag="ps_misc", bufs=2, name=f"pkf{et}_{ic}")
    cr = st.cT[:, :].rearrange("p (nt dt f) -> p dt nt f", nt=8, dt=4)
    for dt in range(4):
        nc.tensor.matmul(
            pk[:, :],
            st.wT["k"][:, (et * 4 + dt) * P:(et * 4 + dt + 1) * P],
            cr[:, dt, 4 * ic:4 * ic + 4, :],
            start=(dt == 0), stop=(dt == 3))
        yield
    nc.vector.tensor_copy(st.kT[et][:, ic * 512:(ic + 1) * 512], pk[:, :])


def _g_cstream(st):
    """Context pairs 1..3: convert + XBAR + kT[0] chunk."""
    nc = st.nc
    for k in range(1, 4):
        sb = st.c_sb[k]
        _conv_xbar_xc(st, "c", k, sb, eng=(nc.gpsimd, nc.gpsimd))
        yield
        yield from _k_proj_chunk(st, 0, k)
        yield


def _g_wv(st):
    yield from _conv_xbar_w(st, "v", st.wv_sb, range(4), st.nc.gpsimd)


def _g_vproj(st):
    """v[mt] = cT(mt)^T @ WvT, one matmul per yield."""
    nc = st.nc
    wr = st.wT["v"][:, :].rearrange("p (et dt f) -> p dt et f", et=4, dt=4)
    for mt in range(8):
        pv = st.ps.tile([P, INNER], F32, tag="ps_misc", bufs=2, name=f"pv{mt}")
        for dt in range(4):
            nc.tensor.matmul(
                pv[:, :],
                st.cT[:, (mt * 4 + dt) * P:(mt * 4 + dt + 1) * P],
                wr[:, dt, :, :],
                start=(dt == 0), stop=(dt == 3))
            if dt == 3:
                nc.vector.tensor_copy(st.v[mt][:, :], pv[:, :])
            yield


def _g_wrest(st):
    """Remaining e-tiles of Wq/Wk (et 1..3)."""
    yield from _conv_xbar_w(st, "q", st.wq_sb, range(1, 4), st.nc.vector)
    yield from _conv_xbar_w(st, "k", st.wk_sb, range(1, 4), st.nc.vector)


def _g_proj(st, et):
    """Full q/k projections for pair et (1..3)."""
    for ic in range(2):
        yield from _q_proj(st, et, ic)
        yield
        yield from _k_proj(st, et, ic)
        yield


def _g_wo(st):
    """Wo load pipeline + bo broadcast."""
    nc = st.nc
    sb = st.ld.tile([P, 2048], F32, tag="wld", bufs=3, name="wold")
    nc.sync.dma_start(
        sb[:, :],
        st.wo_d[:, :].rearrange("(t p) d -> p t d", p=P))
    nc.sync.dma_start(st.bo_s[:, :], st.bo_d[None, :])
    yield
    for dt in range(4):
        bf = st.ld.tile([P, 512], CDT, tag="wbc", bufs=3, name=f"wbo{dt}")
        nc.vector.tensor_copy(bf[:, :], sb[:, dt * 512:(dt + 1) * 512])
        _xbar(nc, _r3(st.woT[:, dt * 512:(dt + 1) * 512], 4), bf[:, :])
        yield
    nc.vector.tensor_copy(st.bo_sb[:, :], st.bo_s[:, :])
    pbo = st.ps.tile([P, D], F32, tag="ps_misc", bufs=2, name="pbo")
    nc.tensor.matmul(pbo[:, :], st.ones128[:, :], st.bo_sb[:, :],
                     start=True, stop=True)
    nc.vector.tensor_copy(st.bo_b[:, :], pbo[:, :])
    yield


def _g_ypair(st, et2, to_dram):
    """Output projection for closed pair et2: y += outT[et2]^T @ WoT."""
    nc = st.nc
    wr = st.woT[:, :].rearrange("p (dt et f) -> p et dt f", dt=4, et=4)
    for nt in range(8):
        py = st.ps.tile([P, D], F32, tag="ps_misc", bufs=2, name=f"py{et2}_{nt}")
        nc.tensor.matmul(py[:, :],
                         st.outT[et2][:, nt * P:(nt + 1) * P],
                         wr[:, et2, :, :],
                         start=True, stop=True)
        if et2 == 0:
            nc.vector.tensor_tensor(st.y_acc[nt][:, :], py[:, :], st.bo_b[:, :], Add)
        elif not to_dram:
            nc.vector.tensor_tensor(st.y_acc[nt][:, :], py[:, :], st.y_acc[nt][:, :], Add)
        else:
            ysb = st.norm.tile([P, D], F32, tag="y", bufs=4, name=f"y{nt}")
            nc.vector.tensor_tensor(ysb[:, :], py[:, :], st.y_acc[nt][:, :], Add)
            nc.sync.dma_start(st.out_d[nt * P:(nt + 1) * P, :], ysb[:, :])
        yield


# ----------------------------------------------------------- head loop ----

def _avstep(st, h, jt, sT, invS, eT, av_ps):
    """invS column + vhs + 16 av/r matmuls for (head h, j-tile jt)."""
    nc = st.nc
    nc.vector.reciprocal(invS[:, jt:jt + 1], sT[:, jt:jt + 1])
    vt = st.head.tile([P, DH + 1], CDT, tag=f"vhs{jt}", bufs=2, name=f"vhs{h}_{jt}")
    nc.gpsimd.tensor_scalar_mul(vt[:, 0:DH], st.v[jt][:, h * DH:(h + 1) * DH],
                                invS[:, jt:jt + 1])
    nc.gpsimd.tensor_copy(vt[:, DH:DH + 1], invS[:, jt:jt + 1])
    # start marks the whole 2KB psum bank pending-zero (lazy zeroing): only
    # the first matmul of the head's bank-group starts, only the last stops;
    # each chunk's first write then overwrites instead of accumulating.
    for it in range(8):
        nc.tensor.matmul(av_ps[:, it * DH:(it + 1) * DH],
                         eT[jt][:, it * P:(it + 1) * P],
                         vt[:, 0:DH],
                         start=(jt == 0 and it == 0), stop=(jt == 7 and it == 7))
        nc.tensor.matmul(st.r_all[:, h * 8 + it:h * 8 + it + 1],
                         eT[jt][:, it * P:(it + 1) * P],
                         vt[:, DH:DH + 1],
                         start=(jt == 0 and it == 0), stop=(jt == 7 and it == 7))


def _norm_head(st, h, av_ps):
    """invr + 8 per-partition normalize multiplies into the pair's avn tiles."""
    nc = st.nc
    iv = st.norm.tile([P, 8], F32, tag="invr", bufs=2, name=f"invr{h}")
    nc.vector.reciprocal(iv[:, :], st.r_all[:, h * 8:(h + 1) * 8])
    st.invr[h] = iv
    et2, s = h // 2, h % 2
    for g in range(2):
        key = (et2, g)
        if key not in st.avn:
            st.avn[key] = st.norm.tile([P, 512], CDT, tag=f"avn{g}", bufs=2,
                                       name=f"avn{et2}_{g}")
        for li in range(4):
            it = g * 4 + li
            # gpsimd cannot read PSUM -> DVE
            nc.vector.tensor_scalar_mul(
                st.avn[key][:, li * P + s * DH: li * P + s * DH + DH],
                av_ps[:, it * DH:(it + 1) * DH],
                iv[:, it:it + 1])


def _xbar_pair(st, et2):
    """XBAR the two normalized [128, 512] halves of pair et2 into outT[et2]."""
    for g in range(2):
        _xbar(st.nc, _r3(st.outT[et2][:, g * 512:(g + 1) * 512], 4),
              st.avn[(et2, g)][:, :])


def _run(st):
    nc = st.nc
    # ---- phase 1: critical prefix (emission order = SP queue order) ----
    wq_sb = _load_w(st, "q")
    wk_sb = _load_w(st, "k")
    x_sb = [_load_xc_pair(st, "x", k) for k in range(4)]
    c_sb = [_load_xc_pair(st, "c", 0)]
    st.wq_sb, st.wk_sb = wq_sb, wk_sb
    # prefix converts + xbars (wq et0, wk et0, x all, c pair0)
    _drain(_conv_xbar_w(st, "q", wq_sb, [0], nc.vector))
    _drain(_conv_xbar_w(st, "k", wk_sb, [0], nc.vector))
    for k in range(4):
        _conv_xbar_xc(st, "x", k, x_sb[k])
    _conv_xbar_xc(st, "c", 0, c_sb[0], eng=(nc.gpsimd, nc.gpsimd))
    # remaining loads issue now, behind the prefix on the SP queue
    c_sb += [_load_xc_pair(st, "c", k) for k in range(1, 4)]
    st.c_sb = c_sb
    wv_sb = _load_w(st, "v")
    st.wv_sb = wv_sb
    # qT[0] (both halves) + kT[0] chunk 0
    _drain(_q_proj(st, 0, 0))
    _drain(_q_proj(st, 0, 1))
    _drain(_k_proj_chunk(st, 0, 0))

    st.fillers = [
        ("cstream", _g_cstream(st)),
        ("wv", _g_wv(st)),
        ("vproj", _g_vproj(st)),
        ("wrest", _g_wrest(st)),
        ("proj1", _g_proj(st, 1)),
        ("wo", _g_wo(st)),
        ("proj2", _g_proj(st, 2)),
        ("proj3", _g_proj(st, 3)),
    ]

    # ---- phase 2: head loop ----
    DEADLINES = {2: ("wrest", "proj1"), 4: ("proj2",), 6: ("proj3",)}
    pend = []  # (h, jt, sT, invS, eT, av_ps) not yet emitted
    for h in range(H):
        for need in DEADLINES.get(h, ()):
            _force(st, need)
        et2, ro = h // 2, (h % 2) * DH
        sT = st.head.tile([P, 8], F32, tag="sT", bufs=2, name=f"sT{h}")
        invS = st.head.tile([P, 8], F32, tag="invS", bufs=2, name=f"invS{h}")
        av_ps = st.ps.tile([P, 512], F32, tag="ps_av", bufs=1, name=f"av{h}")
        eT = []
        for jt in range(8):
            if h == 0 and jt >= 2 and jt % 2 == 0:
                # context pair jt/2 must be emitted before sim(0, jt)
                _force_until(st, "cstream", 1 + 6 * (jt // 2))
            psim = st.ps.tile([P, N], F32, tag="ps_sim", bufs=2, name=f"psim{h}_{jt}")
            for ic in range(2):
                nc.tensor.matmul(
                    psim[:, ic * 512:(ic + 1) * 512],
                    st.kT[et2][ro:ro + DH, jt * P:(jt + 1) * P],
                    st.qT[et2][ro:ro + DH, ic * 512:(ic + 1) * 512],
                    start=True, stop=True)
            e = st.head.tile([P, N], CDT, tag=f"expT{jt}", bufs=2, name=f"expT{h}_{jt}")
            nc.scalar.activation(e[:, :], psim[:, :], Exp, scale=SCALE,
                                 accum_out=sT[:, jt:jt + 1])
            eT.append(e)
            pend.append((h, jt, sT, invS, eT, av_ps))
            # emit due avsteps: lag AV0LAG for head 0, 1 afterwards
            lag = AV0LAG if pend[0][0] == 0 else 1
            while pend and (pend[0][0] < h or jt - pend[0][1] >= lag):
                ph, pjt, psT, pinvS, peT, pav = pend.pop(0)
                if ph == 0:
                    _force(st, "cstream")
                    _force(st, "wv")
                    _force_until(st, "vproj", 4 * (pjt + 1))
                _avstep(st, ph, pjt, psT, pinvS, peT, pav)
                if pjt == 7:
                    _norm_head(st, ph, pav)
                    if ph % 2 == 1:
                        _xbar_pair(st, ph // 2)
                        st.fillers.append(
                            (f"y{ph // 2}", _g_ypair(st, ph // 2, ph == H - 1)))
                lag = AV0LAG if pend and pend[0][0] == 0 else 1
            _budget_drain(st, FB)
    # ---- phase 3: tail ----
    while pend:
        ph, pjt, psT, pinvS, peT, pav = pend.pop(0)
        _avstep(st, ph, pjt, psT, pinvS, peT, pav)
        if pjt == 7:
            _norm_head(st, ph, pav)
            if ph % 2 == 1:
                _xbar_pair(st, ph // 2)
                st.fillers.append(
                    (f"y{ph // 2}", _g_ypair(st, ph // 2, ph == H - 1)))
    for pair in st.fillers:
        _drain(pair[1])


_CACHE = {}


def get_nc():
    if "nc" not in _CACHE:
        # Bacc (not raw Bass): its compile() runs the wait-legalization passes
        # (move_matmul_waits_to_ldweights, generate_event_semaphores) that
        # walrus codegen requires (max 1 sync wait per instruction).
        nc = bacc.Bacc("TRN2", target_bir_lowering=False, num_devices=B)
        build(nc)
        nc.compile()
        _CACHE["nc"] = nc
    return _CACHE["nc"]


def kernel(x, context, Wq, Wk, Wv, Wo, bo):
    nc = get_nc()
    w = {
        "Wq": np.ascontiguousarray(Wq, dtype=np.float32),
        "Wk": np.ascontiguousarray(Wk, dtype=np.float32),
        "Wv": np.ascontiguousarray(Wv, dtype=np.float32),
        "Wo": np.ascontiguousarray(Wo, dtype=np.float32),
        "bo": np.ascontiguousarray(bo, dtype=np.float32),
    }
    in_maps = [
        {"x": np.ascontiguousarray(x[b], dtype=np.float32),
         "context": np.ascontiguousarray(context[b], dtype=np.float32),
         **w}
        for b in range(B)
    ]
    res = run_bass_kernel_spmd(nc, in_maps, core_ids=list(range(B)))
    _CACHE["last"] = res
    return np.stack([res.results[b]["out"] for b in range(B)], axis=0)


# revision 19
# speedup vs baseline: 1.0490x; 1.0490x over previous
"""Slot-attention (softmax over queries + key renormalization) on 8 TRN2 NeuronCores.

Sharding: data-parallel over batch (b=8 -> one batch element per core, no
collectives). Per-core fused kernel, built around:

1. All inputs arrive via gpsimd casting DMAs (f32 DRAM -> bf16 SBUF, half
   the serialized DMA-engine hold of an f32 load, no convert pass).
   Layout transposes go through the DMA XBAR (dma_start(transpose=True))
   for weights and late context pairs; x and the first context pair (the
   critical path to the first exp) are transposed on the otherwise-idle PE
   with a bf16 identity.

2. attn@v runs in the [i, e] orientation with the exp tiles as the
   STATIONARY operand: av[i-tile, 64] += expT[jt][:, i-slice]^T @ vhs[jt],
   full 128-row contraction (vs 64 in the [e, i] orientation) -> half the
   PE cycles. A 1-column matmul per (it, jt) accumulates the renormalizer
   r[i] into a persistent PSUM tile. Normalization attn/r[i] is a
   per-partition scalar multiply (DVE), and the normalized [i, e] tiles
   are XBAR-transposed back to [e, i] for the output projection.

Pipeline per head h, j-tile jt: sim (PE) -> exp w/ S[j] accum (ACT) ->
invS column recip (DVE) + vhs (Pool) -> [one step later] 16 av/r matmuls
(PE). Prep (weight streams, v/q/k projections, closed pairs' output
projections) is interleaved as budget-drained filler generators.
The end-to-end critical path is the serial exp stream on ACT.

Matmul operands bf16, accumulation f32 in PSUM, softmax statistics f32.
"""

import os
import sys

sys.path.insert(0, "/opt/trn_rl_repo")

import numpy as np

import concourse.bass as bass
import concourse.mybir as mybir
import concourse.tile as tile
from concourse import bacc
from concourse.bass_utils import run_bass_kernel_spmd
from concourse.masks import make_identity

B = 8
N = 1024  # queries
M = 1024  # keys
D = 512   # model dim
H = 8
DH = 64
INNER = H * DH
SCALE = DH ** -0.5
P = 128

F32 = mybir.dt.float32
CDT = mybir.dt.bfloat16

AV0LAG = int(os.environ.get("AV0LAG", "4"))
FB = int(os.environ.get("FB", "3"))
_DRAINED = {}

Exp = mybir.ActivationFunctionType.Exp
Mult = mybir.AluOpType.mult
Add = mybir.AluOpType.add


def _r3(ap, a):
    return ap.rearrange("p (a b) -> p a b", a=a)


def build(nc: bass.Bass):
    _DRAINED.clear()
    x_d = nc.declare_dram_parameter("x", [N, D], F32, isOutput=False)
    c_d = nc.declare_dram_parameter("context", [M, D], F32, isOutput=False)
    wq_d = nc.declare_dram_parameter("Wq", [INNER, D], F32, isOutput=False)
    wk_d = nc.declare_dram_parameter("Wk", [INNER, D], F32, isOutput=False)
    wv_d = nc.declare_dram_parameter("Wv", [INNER, D], F32, isOutput=False)
    wo_d = nc.declare_dram_parameter("Wo", [D, INNER], F32, isOutput=False)
    bo_d = nc.declare_dram_parameter("bo", [D], F32, isOutput=False)
    out_d = nc.declare_dram_parameter("out", [N, D], F32, isOutput=True)

    with tile.TileContext(nc) as tc:
        with tc.tile_pool(name="const", bufs=1) as const:
            ident_b = const.tile([P, P], CDT, tag="ident_b")
            make_identity(nc, ident_b[:, :])
            ones128 = const.tile([1, P], CDT, tag="ones128")
            nc.gpsimd.memset(ones128[:, :], 1.0)
            # tiny warmup exp so the ACT table load happens at t~0
            warm = const.tile([1, 2], F32, tag="warm")
            nc.gpsimd.memset(warm[:, :], 0.0)
            nc.scalar.activation(warm[:, 0:1], warm[:, 1:2], Exp, scale=1.0)
            bo_s = const.tile([1, D], F32, tag="bo_s")
            bo_sb = const.tile([1, D], CDT, tag="bo_sb")
            bo_b = const.tile([P, D], F32, tag="bo_b")

            with tc.tile_pool(name="stage", bufs=1) as stage:
                # wT[n][p, (et*4+dt)*128 + f] = W[et*128+f, dt*128+p]
                wT = {n: stage.tile([P, 16 * P], CDT, tag=f"w{n}T", name=f"w{n}T")
                      for n in ("q", "k", "v")}
                # woT[p, (dt*4+et)*128 + f] = Wo[dt*128+f, et*128+p]
                woT = stage.tile([P, 16 * P], CDT, tag="woT")
                # xT_b[p, (nt*4+dt)*128 + f] = x[nt*128+f, dt*128+p]
                xT = stage.tile([P, 32 * P], CDT, tag="xT")
                cT = stage.tile([P, 32 * P], CDT, tag="cT")
                v = [stage.tile([P, INNER], CDT, tag=f"v{t}", name=f"v{t}")
                     for t in range(8)]

                with tc.tile_pool(name="outp", bufs=1) as outp:
                    qT = [outp.tile([P, N], CDT, tag=f"qT{t}", name=f"qT{t}") for t in range(4)]
                    kT = [outp.tile([P, M], CDT, tag=f"kT{t}", name=f"kT{t}") for t in range(4)]
                    outT = [outp.tile([P, N], CDT, tag=f"outT{t}", name=f"outT{t}") for t in range(4)]
                    y_acc = [outp.tile([P, D], F32, tag=f"y_acc{t}", name=f"y_acc{t}")
                             for t in range(8)]

                    with tc.tile_pool(name="head", bufs=1) as head, \
                         tc.tile_pool(name="norm", bufs=1) as norm, \
                         tc.tile_pool(name="ld", bufs=1) as ld:
                        ps_at = tc.alloc_tile_pool(name="ps_at", bufs=1, space="PSUM")
                        ps_rp = tc.alloc_tile_pool(name="ps_rp", bufs=1, space="PSUM")
                        r_all = ps_rp.tile([P, 64], F32, tag="r_all")
                        st = _State(nc, tc, head, norm, ld, ps_at, r_all,
                                    wT, woT, xT, cT, v, qT, kT, outT, y_acc,
                                    ident_b, ones128, bo_s, bo_sb, bo_b,
                                    x_d, c_d, wq_d, wk_d, wv_d, wo_d, bo_d, out_d)
                        _run(st)
                        ps_rp.release()
                        ps_at.release()
    return nc


class _State:
    def __init__(self, nc, tc, head, norm, ld, ps_at, r_all,
                 wT, woT, xT, cT, v, qT, kT, outT, y_acc,
                 ident_b, ones128, bo_s, bo_sb, bo_b,
                 x_d, c_d, wq_d, wk_d, wv_d, wo_d, bo_d, out_d):
        self.nc = nc
        self.tc = tc
        self.head = head
        self.norm = norm
        self.ld = ld
        self.ps = ps_at
        self.r_all = r_all
        self.wT = wT
        self.woT = woT
        self.xT = xT
        self.cT = cT
        self.v = v
        self.qT = qT
        self.kT = kT
        self.outT = outT
        self.y_acc = y_acc
        self.ident_b = ident_b
        self.ones128 = ones128
        self.bo_s = bo_s
        self.bo_sb = bo_sb
        self.bo_b = bo_b
        self.x_d = x_d
        self.c_d = c_d
        self.w_d = {"q": wq_d, "k": wk_d, "v": wv_d}
        self.wo_d = wo_d
        self.bo_d = bo_d
        self.out_d = out_d
        self.fillers = []
        self.invr = {}
        self.avn = {}


def _drain(g):
    if g is not None:
        for _ in g:
            pass


def _budget_drain(st, budget):
    while budget > 0 and st.fillers:
        try:
            next(st.fillers[0][1])
            _DRAINED[st.fillers[0][0]] = _DRAINED.get(st.fillers[0][0], 0) + 1
            budget -= 1
        except StopIteration:
            st.fillers.pop(0)


def _force(st, name):
    for pair in list(st.fillers):
        if pair[0] == name:
            _drain(pair[1])
            st.fillers.remove(pair)


def _force_until(st, name, count):
    while st.fillers and st.fillers[0][0] == name and \
            _DRAINED.get(name, 0) < count:
        try:
            next(st.fillers[0][1])
            _DRAINED[name] = _DRAINED.get(name, 0) + 1
        except StopIteration:
            st.fillers.pop(0)


# ---------------------------------------------------------------- prep ----

def _xbar(nc, dst3, src2):
    nc.sync.dma_start(dst3, src2, transpose=True)


def _cast_load(st, name, dram_ap, cols, tag, bufs):
    sb = st.ld.tile([P, cols], CDT, tag=tag, bufs=bufs, name=name)
    st.nc.gpsimd.dma_start(sb[:, :], dram_ap)
    return sb


def _pe_xpose_tile(st, which, nt, src, col_off):
    """PE-transpose one [128, 512] source tile (4 chunks) into xT_b/cT_b."""
    nc = st.nc
    dstT = st.xT if which == "x" else st.cT
    tp = st.ps.tile([P, 512], CDT, tag="ps_misc", bufs=2, name=f"tp{which}{nt}")
    for dt in range(4):
        nc.tensor.transpose(tp[:, dt * P:(dt + 1) * P],
                            src[:, col_off + dt * P:col_off + (dt + 1) * P],
                            st.ident_b[:, :])
    nc.vector.tensor_copy(dstT[:, (nt * 4) * P:(nt * 4 + 4) * P], tp[:, :])


def _q_proj(st, et, ic):
    nc = st.nc
    pp = st.ps.tile([P, 512], F32, tag="ps_misc", bufs=2, name=f"pq{et}_{ic}")
    xr = st.xT[:, :].rearrange("p (nt dt f) -> p dt nt f", nt=8, dt=4)
    for dt in range(4):
        nc.tensor.matmul(
            pp[:, :],
            st.wT["q"][:, (et * 4 + dt) * P:(et * 4 + dt + 1) * P],
            xr[:, dt, 4 * ic:4 * ic + 4, :],
            start=(dt == 0), stop=(dt == 3))
        yield
    nc.vector.tensor_copy(st.qT[et][:, ic * 512:(ic + 1) * 512], pp[:, :])


def _k_proj_chunk(st, et, k):
    nc = st.nc
    pk = st.ps.tile([P, 256], F32, tag="ps_misc", bufs=2, name=f"pk{et}_{k}")
    cr = st.cT[:, :].rearrange("p (nt dt f) -> p dt nt f", nt=8, dt=4)
    for dt in range(4):
        nc.tensor.matmul(
            pk[:, :],
            st.wT["k"][:, (et * 4 + dt) * P:(et * 4 + dt + 1) * P],
            cr[:, dt, 2 * k:2 * k + 2, :],
            start=(dt == 0), stop=(dt == 3))
        yield
    nc.vector.tensor_copy(st.kT[et][:, k * 256:(k + 1) * 256], pk[:, :])


def _k_proj(st, et, ic):
    nc = st.nc
    pk = st.ps.tile([P, 512], F32, tag="ps_misc", bufs=2, name=f"pkf{et}_{ic}")
    cr = st.cT[:, :].rearrange("p (nt dt f) -> p dt nt f", nt=8, dt=4)
    for dt in range(4):
        nc.tensor.matmul(
            pk[:, :],
            st.wT["k"][:, (et * 4 + dt) * P:(et * 4 + dt + 1) * P],
            cr[:, dt, 4 * ic:4 * ic + 4, :],
            start=(dt == 0), stop=(dt == 3))
        yield
    nc.vector.tensor_copy(st.kT[et][:, ic * 512:(ic + 1) * 512], pk[:, :])


def _g_cstream(st):
    """Context pairs 1..3: XBAR from the c123 staging tile + kT[0] chunk."""
    for k in range(1, 4):
        _xbar(st.nc, _r3(st.cT[:, 8 * k * P:(8 * k + 8) * P], 8),
              st.c123_sb[:, (k - 1) * 1024:k * 1024])
        yield
        yield from _k_proj_chunk(st, 0, k)
        yield


def _g_wv(st):
    for et in range(4):
        _xbar(st.nc, _r3(st.wT["v"][:, et * 512:(et + 1) * 512], 4),
              st.wv_sb[:, et * 512:(et + 1) * 512])
        yield


def _g_vproj(st):
    nc = st.nc
    wr = st.wT["v"][:, :].rearrange("p (et dt f) -> p dt et f", et=4, dt=4)
    for mt in range(8):
        pv = st.ps.tile([P, INNER], F32, tag="ps_misc", bufs=2, name=f"pv{mt}")
        for dt in range(4):
            nc.tensor.matmul(
                pv[:, :],
                st.cT[:, (mt * 4 + dt) * P:(mt * 4 + dt + 1) * P],
                wr[:, dt, :, :],
                start=(dt == 0), stop=(dt == 3))
            if dt == 3:
                nc.vector.tensor_copy(st.v[mt][:, :], pv[:, :])
            yield


def _g_wrest(st):
    """Wq/Wk e-tiles 1..3: et1 from the 01 staging, et2/3 from new loads."""
    nc = st.nc
    for name in ("q", "k"):
        sb01 = st.w01_sb[name]
        _xbar(nc, _r3(st.wT[name][:, 512:1024], 4), sb01[:, 512:1024])
        yield
        sb23 = _cast_load(
            st, f"w{name}23",
            st.w_d[name][2 * P:4 * P, :].rearrange("(t p) d -> p t d", p=P),
            1024, "wld", 4)
        for et in (2, 3):
            _xbar(nc, _r3(st.wT[name][:, et * 512:(et + 1) * 512], 4),
                  sb23[:, (et - 2) * 512:(et - 1) * 512])
            yield


def _g_proj(st, et):
    for ic in range(2):
        yield from _q_proj(st, et, ic)
        yield
        yield from _k_proj(st, et, ic)
        yield


def _g_wo(st):
    nc = st.nc
    sb = _cast_load(st, "wold",
                    st.wo_d[:, :].rearrange("(t p) d -> p t d", p=P),
                    2048, "wld2", 2)
    nc.sync.dma_start(st.bo_s[:, :], st.bo_d[None, :])
    yield
    for dt in range(4):
        _xbar(nc, _r3(st.woT[:, dt * 512:(dt + 1) * 512], 4),
              sb[:, dt * 512:(dt + 1) * 512])
        yield
    nc.vector.tensor_copy(st.bo_sb[:, :], st.bo_s[:, :])
    pbo = st.ps.tile([P, D], F32, tag="ps_misc", bufs=2, name="pbo")
    nc.tensor.matmul(pbo[:, :], st.ones128[:, :], st.bo_sb[:, :],
                     start=True, stop=True)
    nc.vector.tensor_copy(st.bo_b[:, :], pbo[:, :])
    yield


def _g_ypair(st, et2, to_dram):
    nc = st.nc
    wr = st.woT[:, :].rearrange("p (dt et f) -> p et dt f", dt=4, et=4)
    for nt in range(8):
        py = st.ps.tile([P, D], F32, tag="ps_misc", bufs=2, name=f"py{et2}_{nt}")
        nc.tensor.matmul(py[:, :],
                         st.outT[et2][:, nt * P:(nt + 1) * P],
                         wr[:, et2, :, :],
                         start=True, stop=True)
        if et2 == 0:
            nc.vector.tensor_tensor(st.y_acc[nt][:, :], py[:, :], st.bo_b[:, :], Add)
        elif not to_dram:
            nc.vector.tensor_tensor(st.y_acc[nt][:, :], py[:, :], st.y_acc[nt][:, :], Add)
        else:
            ysb = st.norm.tile([P, D], F32, tag="y", bufs=4, name=f"y{nt}")
            nc.vector.tensor_tensor(ysb[:, :], py[:, :], st.y_acc[nt][:, :], Add)
            nc.sync.dma_start(st.out_d[nt * P:(nt + 1) * P, :], ysb[:, :])
        yield


# ----------------------------------------------------------- head loop ----

def _prep_vhs(st, h, jt, sT, invS, vhs_list):
    """invS column reciprocal (DVE) + vhs tile build (Pool)."""
    nc = st.nc
    nc.vector.reciprocal(invS[:, jt:jt + 1], sT[:, jt:jt + 1])
    vt = st.head.tile([P, DH + 1], CDT, tag=f"vhs{jt}", bufs=2, name=f"vhs{h}_{jt}")
    nc.gpsimd.tensor_scalar_mul(vt[:, 0:DH], st.v[jt][:, h * DH:(h + 1) * DH],
                                invS[:, jt:jt + 1])
    nc.gpsimd.tensor_copy(vt[:, DH:DH + 1], invS[:, jt:jt + 1])
    vhs_list.append(vt)


def _avstep_mm(st, h, jt, eT, vhs_list, av_ps):
    """16 av/r matmuls for (head h, j-tile jt)."""
    nc = st.nc
    vt = vhs_list[jt]
    # start marks the whole 2KB psum bank pending-zero (lazy zeroing): only
    # the first matmul of the head's bank-group starts, only the last stops;
    # each chunk's first write then overwrites instead of accumulating.
    for it in range(8):
        nc.tensor.matmul(av_ps[:, it * DH:(it + 1) * DH],
                         eT[jt][:, it * P:(it + 1) * P],
                         vt[:, 0:DH],
                         start=(jt == 0 and it == 0), stop=(jt == 7 and it == 7))
        nc.tensor.matmul(st.r_all[:, h * 8 + it:h * 8 + it + 1],
                         eT[jt][:, it * P:(it + 1) * P],
                         vt[:, DH:DH + 1],
                         start=(jt == 0 and it == 0), stop=(jt == 7 and it == 7))


def _invr(st, h):
    iv = st.norm.tile([P, 8], F32, tag="invr", bufs=2, name=f"invr{h}")
    st.nc.vector.reciprocal(iv[:, :], st.r_all[:, h * 8:(h + 1) * 8])
    st.invr[h] = iv


def _norm_head_g(st, h, av_ps, g):
    """Per-partition normalize of 4 i-tiles into the pair's avn[g] tile."""
    nc = st.nc
    iv = st.invr[h]
    et2, s = h // 2, h % 2
    key = (et2, g)
    if key not in st.avn:
        st.avn[key] = st.norm.tile([P, 512], CDT, tag=f"avn{g}", bufs=2,
                                   name=f"avn{et2}_{g}")
    for li in range(4):
        it = g * 4 + li
        nc.vector.tensor_scalar_mul(
            st.avn[key][:, li * P + s * DH: li * P + s * DH + DH],
            av_ps[:, it * DH:(it + 1) * DH],
            iv[:, it:it + 1])


def _xbar_pair_g(st, et2, g):
    _xbar(st.nc, _r3(st.outT[et2][:, g * 512:(g + 1) * 512], 4),
          st.avn[(et2, g)][:, :])


def _close_head(st, h, av_ps):
    """invr + normalizes; on odd heads queue the pair's XBARs + y-proj."""
    _invr(st, h)
    for g in range(2):
        _norm_head_g(st, h, av_ps, g)
        if h % 2 == 1:
            _xbar_pair_g(st, h // 2, g)
    if h % 2 == 1:
        st.fillers.append((f"y{h // 2}", _g_ypair(st, h // 2, h == H - 1)))


def _run(st):
    nc = st.nc
    # ---- phase 1: critical prefix ----
    st.w01_sb = {}
    for name in ("q", "k"):
        st.w01_sb[name] = _cast_load(
            st, f"w{name}01",
            st.w_d[name][0:2 * P, :].rearrange("(t p) d -> p t d", p=P),
            1024, "wld", 4)
    x_sb = _cast_load(st, "xall",
                      st.x_d[:, :].rearrange("(t p) d -> p t d", p=P),
                      4096, "xld", 1)
    c0_sb = _cast_load(st, "c0ld",
                       st.c_d[0:2 * P, :].rearrange("(t p) d -> p t d", p=P),
                       1024, "cld0", 1)
    # et0 weight XBARs
    _xbar(nc, _r3(st.wT["q"][:, 0:512], 4), st.w01_sb["q"][:, 0:512])
    _xbar(nc, _r3(st.wT["k"][:, 0:512], 4), st.w01_sb["k"][:, 0:512])
    # x + context pair 0 through the PE
    for nt in range(8):
        _pe_xpose_tile(st, "x", nt, x_sb, nt * 512)
    for nt in range(2):
        _pe_xpose_tile(st, "c", nt, c0_sb, nt * 512)
    # qT[0] + kT[0] chunk 0
    _drain(_q_proj(st, 0, 0))
    _drain(_q_proj(st, 0, 1))
    _drain(_k_proj_chunk(st, 0, 0))
    # remaining big loads (issue order on the Pool SWDGE queue)
    st.c123_sb = _cast_load(
        st, "c123",
        st.c_d[2 * P:8 * P, :].rearrange("(t p) d -> p t d", p=P),
        3072, "cld123", 1)
    st.wv_sb = _cast_load(st, "wvld",
                          st.w_d["v"][:, :].rearrange("(t p) d -> p t d", p=P),
                          2048, "wld2", 2)

    st.fillers = [
        ("cstream", _g_cstream(st)),
        ("wv", _g_wv(st)),
        ("vproj", _g_vproj(st)),
        ("wrest", _g_wrest(st)),
        ("proj1", _g_proj(st, 1)),
        ("wo", _g_wo(st)),
        ("proj2", _g_proj(st, 2)),
        ("proj3", _g_proj(st, 3)),
    ]

    # ---- phase 2: head loop ----
    DEADLINES = {2: ("wrest", "proj1"), 4: ("proj2",), 6: ("proj3",)}
    pend = []  # (h, jt, eT, vhs_list, av_ps)
    for h in range(H):
        for need in DEADLINES.get(h, ()):
            _force(st, need)
        et2, ro = h // 2, (h % 2) * DH
        sT = st.head.tile([P, 8], F32, tag="sT", bufs=2, name=f"sT{h}")
        invS = st.head.tile([P, 8], F32, tag="invS", bufs=2, name=f"invS{h}")
        av_ps = st.ps.tile([P, 512], F32, tag="ps_av", bufs=1, name=f"av{h}")
        eT = []
        vhs_list = []
        for jt in range(8):
            if h == 0 and jt >= 2 and jt % 2 == 0:
                _force_until(st, "cstream", 6 * (jt // 2))
            psim = st.ps.tile([P, N], F32, tag="ps_sim", bufs=2, name=f"psim{h}_{jt}")
            for ic in range(2):
                nc.tensor.matmul(
                    psim[:, ic * 512:(ic + 1) * 512],
                    st.kT[et2][ro:ro + DH, jt * P:(jt + 1) * P],
                    st.qT[et2][ro:ro + DH, ic * 512:(ic + 1) * 512],
                    start=True, stop=True)
            e = st.head.tile([P, N], CDT, tag=f"expT{jt}", bufs=2, name=f"expT{h}_{jt}")
            nc.scalar.activation(e[:, :], psim[:, :], Exp, scale=SCALE,
                                 accum_out=sT[:, jt:jt + 1])
            eT.append(e)
            if h >= 1:
                _prep_vhs(st, h, jt, sT, invS, vhs_list)
            pend.append((h, jt, eT, vhs_list, av_ps))
            lag = AV0LAG if pend[0][0] == 0 else 1
            while pend and (pend[0][0] < h or jt - pend[0][1] >= lag):
                ph, pjt, peT, pvhs, pav = pend.pop(0)
                if ph == 0:
                    _force(st, "cstream")
                    _force(st, "wv")
                    _force_until(st, "vproj", 4 * (pjt + 1))
                    _prep_vhs(st, ph, pjt, sT if ph == h else psT0, invS if ph == h else pinvS0, pvhs)
                _avstep_mm(st, ph, pjt, peT, pvhs, pav)
                if pjt == 7:
                    _close_head(st, ph, pav)
                lag = AV0LAG if pend and pend[0][0] == 0 else 1
            _budget_drain(st, FB)
        if h == 0:
            psT0, pinvS0 = sT, invS
    # ---- phase 3: tail ----
    assert len(pend) == 1
    ph, pjt, peT, pvhs, pav = pend.pop(0)
    _avstep_mm(st, ph, pjt, peT, pvhs, pav)
    _invr(st, ph)
    yg = _g_ypair(st, ph // 2, True)
    for g in range(2):
        _norm_head_g(st, ph, pav, g)
        _xbar_pair_g(st, ph // 2, g)
        for _ in range(4):
            next(yg, None)
    _drain(yg)
    for pair in st.fillers:
        _drain(pair[1])


_CACHE = {}


def get_nc():
    if "nc" not in _CACHE:
        # Bacc (not raw Bass): its compile() runs the wait-legalization passes
        # (move_matmul_waits_to_ldweights, generate_event_semaphores) that
        # walrus codegen requires (max 1 sync wait per instruction).
        nc = bacc.Bacc("TRN2", target_bir_lowering=False, num_devices=B)
        build(nc)
        nc.compile()
        _CACHE["nc"] = nc
    return _CACHE["nc"]


def kernel(x, context, Wq, Wk, Wv, Wo, bo):
    nc = get_nc()
    w = {
        "Wq": np.ascontiguousarray(Wq, dtype=np.float32),
        "Wk": np.ascontiguousarray(Wk, dtype=np.float32),
        "Wv": np.ascontiguousarray(Wv, dtype=np.float32),
        "Wo": np.ascontiguousarray(Wo, dtype=np.float32),
        "bo": np.ascontiguousarray(bo, dtype=np.float32),
    }
    in_maps = [
        {"x": np.ascontiguousarray(x[b], dtype=np.float32),
         "context": np.ascontiguousarray(context[b], dtype=np.float32),
         **w}
        for b in range(B)
    ]
    res = run_bass_kernel_spmd(nc, in_maps, core_ids=list(range(B)))
    _CACHE["last"] = res
    return np.stack([res.results[b]["out"] for b in range(B)], axis=0)
